# revision 4
# baseline (speedup 1.0000x reference)
"""Segmented (block-diagonal per-irrep) linear layer on 8 TRN2 NeuronCores.

Strategy: data-parallel over rows (N=16384 -> 2048/core). Host pre-transposes
x into a channel-major blocked layout so every device matmul is natural:
  yT[v, n] = sum_u Wseg[u, v] * xT[u, n]   (per irrep component)
with weights stationary [K=u, M=v], x moving [K=u, N=n], fp16 storage/compute
(scale 1/sqrt(mul) pre-folded into weights in fp32), fp32 PSUM accumulation.
"""
import sys

sys.path.insert(0, "/opt/trn_rl_repo")

import numpy as np

IRREPS = [(512, 1), (256, 3), (128, 5)]
N_TOTAL = 16384
N_CORES = 8
NC_N = N_TOTAL // N_CORES          # 2048 rows per core
DIM = 1920
NCHUNK = 512                        # matmul moving free dim
P = 128

_runner = None


def _chunked_drain_tile_context(tile, mybir, max_waits=1):
    """TileContext whose final drain splits sem waits across nops.

    The walrus build in this container rejects >2 sync waits on one
    instruction ("Too many sync wait commands"); stock Tile attaches every
    outstanding sem wait to the single kernel-tail Drain. Equivalent
    semantics: chain of same-queue nops each carrying <=2 waits.
    """
    from concourse.vector_clock import ScopedClock

    class ChunkedDrainTileContext(tile.TileContext):
        def _drain_and_barrier(self, tick_clock, wait_clock):
            probe = self.nc.sync.nop()
            wait_clock.add_sem_waits(
                probe.ins, ScopedClock({None: tick_clock.global_clock})
            )
            waits = list(probe.ins.sync_info.on_wait) if probe.ins.sync_info else []
            probe.ins.sync_info = mybir.SyncInfo(
                on_wait=waits[:max_waits], on_update=[]
            )
            for i in range(max_waits, len(waits), max_waits):
                n = self.nc.sync.nop()
                n.ins.sync_info = mybir.SyncInfo(
                    on_wait=waits[i : i + max_waits], on_update=[]
                )
            self.nc.sync.drain()
            self.nc.all_engine_barrier()
            assert self.sems is not None
            popped = self.nc._tile_sem_poison_stack.pop()
            assert popped is self._sem_poison
            self.nc.clear_and_free_semaphores(list(self.sems.allocated().values()))
            self.nc.all_engine_barrier()

    return ChunkedDrainTileContext


def _split_multiwait(nc, mybir, max_waits=1):
    """Walrus in this container rejects >2 sync waits per instruction.

    Move excess waits onto freshly inserted NoOps just before the
    instruction on the same engine queue — identical sync semantics.
    """
    seq = 0
    for f in nc.m.functions:
        for blk in f.blocks:
            changed = False
            new = []
            for inst in blk.instructions:
                si = inst.sync_info
                waits = list(si.on_wait) if si else []
                if len(waits) > max_waits:
                    changed = True
                    updates = list(si.on_update)
                    extra = waits[:-max_waits]
                    for i in range(0, len(extra), max_waits):
                        nop = mybir.InstNoOp(
                            name=f"I-waitsplit-{seq}", ins=[], outs=[]
                        )
                        seq += 1
                        nop.engine = inst.engine
                        nop.sync_info = mybir.SyncInfo(
                            on_wait=extra[i : i + max_waits], on_update=[]
                        )
                        new.append(nop)
                    inst.sync_info = mybir.SyncInfo(
                        on_wait=waits[-max_waits:], on_update=updates
                    )
                new.append(inst)
            if changed:
                blk.instructions = new


def _build_nc():
    import concourse.bass as bass
    import concourse.tile as tile
    from concourse import mybir

    f16 = mybir.dt.float16
    f32 = mybir.dt.float32

    nc = bass.Bass()
    XT = nc.declare_dram_parameter("xt", [DIM, NC_N], f16, isOutput=False)
    W0 = nc.declare_dram_parameter("w0", [512, 512], f16, isOutput=False)
    W1 = nc.declare_dram_parameter("w1", [256, 256], f16, isOutput=False)
    W2 = nc.declare_dram_parameter("w2", [128, 128], f16, isOutput=False)
    YT = nc.declare_dram_parameter("yt", [DIM, NC_N], f16, isOutput=True)

    TC = _chunked_drain_tile_context(tile, mybir)
    n_nchunks = NC_N // NCHUNK

    with TC(nc) as tc:
        with (
            tc.tile_pool(name="w", bufs=1) as wpool,
            tc.tile_pool(name="x", bufs=1) as xpool,
            tc.tile_pool(name="o", bufs=6) as opool,
            tc.tile_pool(name="ps", bufs=8, space="PSUM") as pspool,
        ):
            # --- weights: resident, one tile per 128-row u-chunk ---
            w0t = []
            for uc in range(4):
                t = wpool.tile([P, 512], f16, tag=f"w0_{uc}")
                nc.sync.dma_start(out=t[:], in_=W0[uc * P : (uc + 1) * P, :])
                w0t.append(t)
            w1t = []
            for uc in range(2):
                t = wpool.tile([P, 256], f16, tag=f"w1_{uc}")
                nc.sync.dma_start(out=t[:], in_=W1[uc * P : (uc + 1) * P, :])
                w1t.append(t)
            w2t = wpool.tile([P, 128], f16, tag="w2")
            nc.sync.dma_start(out=w2t[:], in_=W2[:, :])

            # --- x: resident, one tile per 128-row block of xt (15 blocks) ---
            xtiles = []
            for b in range(DIM // P):
                t = xpool.tile([P, NC_N], f16, tag=f"x{b}")
                nc.sync.dma_start(out=t[:], in_=XT[b * P : (b + 1) * P, :])
                xtiles.append(t)

            def emit(out_row, w_tiles, w_col0, x_blocks):
                """yt[out_row:out_row+128] (over all n-chunks) =
                sum_k w_tiles[k][:, w_col0:w_col0+128].T @ x_blocks[k]"""
                for j in range(n_nchunks):
                    ps = pspool.tile([P, NCHUNK], f32, tag="ps")
                    nk = len(x_blocks)
                    for k in range(nk):
                        nc.tensor.matmul(
                            ps[:],
                            w_tiles[k][:, w_col0 : w_col0 + P],
                            x_blocks[k][:, j * NCHUNK : (j + 1) * NCHUNK],
                            start=(k == 0),
                            stop=(k == nk - 1),
                        )
                    ot = opool.tile([P, NCHUNK], f16, tag="o")
                    nc.vector.tensor_copy(out=ot[:], in_=ps[:])
                    nc.sync.dma_start(
                        out=YT[
                            out_row : out_row + P,
                            j * NCHUNK : (j + 1) * NCHUNK,
                        ],
                        in_=ot[:],
                    )

            # segment 0: rows v in [0, 512)
            for vc in range(4):
                emit(vc * P, w0t, vc * P, xtiles[0:4])
            # segment 1: rows 512 + i*256 + v
            for i in range(3):
                xb = xtiles[4 + 2 * i : 4 + 2 * i + 2]
                for vc in range(2):
                    emit(512 + i * 256 + vc * P, w1t, vc * P, xb)
            # segment 2: rows 1280 + i*128 + v
            for i in range(5):
                emit(1280 + i * P, [w2t], 0, [xtiles[10 + i]])

    _split_multiwait(nc, mybir)
    return nc


class _SpmdRunner:
    def __init__(self, nc, n_cores):
        import jax
        from jax.sharding import Mesh, PartitionSpec
        from jax.experimental.shard_map import shard_map
        from concourse import mybir
        from concourse.bass2jax import (
            _bass_exec_p,
            install_neuronx_cc_hook,
            partition_id_tensor,
        )

        install_neuronx_cc_hook()
        self.jax = jax
        self.n_cores = n_cores
        partition_name = (
            nc.partition_id_tensor.name if nc.partition_id_tensor else None
        )
        in_names, out_names, out_avals = [], [], []
        for alloc in nc.m.functions[0].allocations:
            if not isinstance(alloc, mybir.MemoryLocationSet):
                continue
            name = alloc.memorylocations[0].name
            if alloc.kind == "ExternalInput":
                if name != partition_name:
                    in_names.append(name)
            elif alloc.kind == "ExternalOutput":
                out_names.append(name)
                out_avals.append(
                    jax.core.ShapedArray(
                        tuple(alloc.tensor_shape), mybir.dt.np(alloc.dtype)
                    )
                )
        self.in_names = in_names
        self.out_names = out_names
        self.out_avals = out_avals
        self.n_params = len(in_names)
        all_in_names = in_names + out_names
        if partition_name is not None:
            all_in_names = all_in_names + [partition_name]

        def _body(*args):
            operands = list(args)
            if partition_name is not None:
                operands.append(partition_id_tensor())
            outs = _bass_exec_p.bind(
                *operands,
                out_avals=tuple(out_avals),
                in_names=tuple(all_in_names),
                out_names=tuple(out_names),
                lowering_input_output_aliases=(),
                sim_require_finite=True,
                sim_require_nnan=True,
                nc=nc,
            )
            return tuple(outs)

        devices = jax.devices()[:n_cores]
        self.mesh = Mesh(np.asarray(devices), ("core",))
        n_args = self.n_params + len(out_names)
        self.fn = jax.jit(
            shard_map(
                _body,
                mesh=self.mesh,
                in_specs=(PartitionSpec("core"),) * n_args,
                out_specs=(PartitionSpec("core"),) * len(out_names),
                check_rep=False,
            ),
            keep_unused=True,
        )
        self._dev_args = None

    def set_inputs(self, in_maps):
        import jax
        from jax.sharding import PartitionSpec

        per_core = [[np.asarray(m[name]) for name in self.in_names] for m in in_maps]
        concat_in = [
            np.concatenate([per_core[c][i] for c in range(self.n_cores)], axis=0)
            for i in range(self.n_params)
        ]
        concat_zeros = [
            np.zeros((self.n_cores * a.shape[0], *a.shape[1:]), a.dtype)
            for a in self.out_avals
        ]
        sharding = jax.sharding.NamedSharding(self.mesh, PartitionSpec("core"))
        self._dev_args = [
            jax.device_put(a, sharding) for a in (*concat_in, *concat_zeros)
        ]

    def run_raw(self):
        return self.fn(*self._dev_args)

    def run(self):
        out_arrs = self.jax.block_until_ready(self.run_raw())
        return [
            {
                name: np.asarray(out_arrs[i]).reshape(
                    self.n_cores, *self.out_avals[i].shape
                )[c]
                for i, name in enumerate(self.out_names)
            }
            for c in range(self.n_cores)
        ]


def _get_runner():
    global _runner
    if _runner is None:
        _runner = _SpmdRunner(_build_nc(), N_CORES)
    return _runner


def _pack_x(x):
    """[N, 1920] f32 -> blocked channel-major [1920, N] f16."""
    n = x.shape[0]
    x0 = x[:, :512].T
    x1 = x[:, 512:1280].reshape(n, 256, 3).transpose(2, 1, 0).reshape(768, n)
    x2 = x[:, 1280:1920].reshape(n, 128, 5).transpose(2, 1, 0).reshape(640, n)
    return np.concatenate([x0, x1, x2], axis=0).astype(np.float16)


def _unpack_y(yt):
    """blocked [1920, N] f16 -> [N, 1920] f32."""
    n = yt.shape[1]
    y0 = yt[:512].T
    y1 = yt[512:1280].reshape(3, 256, n).transpose(2, 1, 0).reshape(n, 768)
    y2 = yt[1280:1920].reshape(5, 128, n).transpose(2, 1, 0).reshape(n, 640)
    return np.concatenate([y0, y1, y2], axis=1).astype(np.float32)


def _pack_weights(weight):
    w = np.asarray(weight, dtype=np.float32)
    out = {}
    off = 0
    for idx, (mul, _d) in enumerate(IRREPS):
        blk = w[off : off + mul * mul].reshape(mul, mul) / np.sqrt(np.float32(mul))
        out[f"w{idx}"] = blk.astype(np.float16)
        off += mul * mul
    return out


def kernel(x, weight):
    x = np.asarray(x)
    runner = _get_runner()
    xt = _pack_x(x)
    wmap = _pack_weights(weight)
    in_maps = []
    for c in range(N_CORES):
        m = {"xt": np.ascontiguousarray(xt[:, c * NC_N : (c + 1) * NC_N])}
        m.update(wmap)
        in_maps.append(m)
    runner.set_inputs(in_maps)
    results = runner.run()
    yt = np.concatenate([results[c]["yt"] for c in range(N_CORES)], axis=1)
    return _unpack_y(yt)


# revision 9
# speedup vs baseline: 1.0214x; 1.0214x over previous
"""Segmented (block-diagonal per-irrep) linear layer on 8 TRN2 NeuronCores.

Strategy: data-parallel over rows (N=16384 -> 2048/core). Host pre-transposes
x into a channel-major blocked layout so every device matmul is natural:
  yT[v, n] = sum_u Wseg[u, v] * xT[u, n]   (per irrep component)
with weights stationary [K=u, M=v], x moving [K=u, N=n], fp16 storage/compute
(scale 1/sqrt(mul) pre-folded into weights in fp32), fp32 PSUM accumulation.
"""
import sys

sys.path.insert(0, "/opt/trn_rl_repo")

import numpy as np

IRREPS = [(512, 1), (256, 3), (128, 5)]
N_TOTAL = 16384
N_CORES = 8
NC_N = N_TOTAL // N_CORES          # 2048 rows per core
DIM = 1920
NCHUNK = 512                        # matmul moving free dim
P = 128

_runner = None


def _chunked_drain_tile_context(tile, mybir, max_waits=1):
    """TileContext whose final drain splits sem waits across nops.

    The walrus build in this container rejects >2 sync waits on one
    instruction ("Too many sync wait commands"); stock Tile attaches every
    outstanding sem wait to the single kernel-tail Drain. Equivalent
    semantics: chain of same-queue nops each carrying <=2 waits.
    """
    from concourse.vector_clock import ScopedClock

    class ChunkedDrainTileContext(tile.TileContext):
        def _drain_and_barrier(self, tick_clock, wait_clock):
            probe = self.nc.sync.nop()
            wait_clock.add_sem_waits(
                probe.ins, ScopedClock({None: tick_clock.global_clock})
            )
            waits = list(probe.ins.sync_info.on_wait) if probe.ins.sync_info else []
            probe.ins.sync_info = mybir.SyncInfo(
                on_wait=waits[:max_waits], on_update=[]
            )
            for i in range(max_waits, len(waits), max_waits):
                n = self.nc.sync.nop()
                n.ins.sync_info = mybir.SyncInfo(
                    on_wait=waits[i : i + max_waits], on_update=[]
                )
            self.nc.sync.drain()
            self.nc.all_engine_barrier()
            assert self.sems is not None
            popped = self.nc._tile_sem_poison_stack.pop()
            assert popped is self._sem_poison
            self.nc.clear_and_free_semaphores(list(self.sems.allocated().values()))
            self.nc.all_engine_barrier()

    return ChunkedDrainTileContext


def _split_multiwait(nc, mybir, max_waits=1):
    """Walrus in this container rejects >2 sync waits per instruction.

    Move excess waits onto freshly inserted NoOps just before the
    instruction on the same engine queue — identical sync semantics.
    """
    seq = 0
    for f in nc.m.functions:
        for blk in f.blocks:
            changed = False
            new = []
            for inst in blk.instructions:
                si = inst.sync_info
                waits = list(si.on_wait) if si else []
                if len(waits) > max_waits:
                    changed = True
                    updates = list(si.on_update)
                    extra = waits[:-max_waits]
                    for i in range(0, len(extra), max_waits):
                        nop = mybir.InstNoOp(
                            name=f"I-waitsplit-{seq}", ins=[], outs=[]
                        )
                        seq += 1
                        nop.engine = inst.engine
                        nop.sync_info = mybir.SyncInfo(
                            on_wait=extra[i : i + max_waits], on_update=[]
                        )
                        new.append(nop)
                    inst.sync_info = mybir.SyncInfo(
                        on_wait=waits[-max_waits:], on_update=updates
                    )
                new.append(inst)
            if changed:
                blk.instructions = new


def _build_nc(reps=1):
    import concourse.bass as bass
    import concourse.tile as tile
    from concourse import mybir

    f16 = mybir.dt.float16
    f32 = mybir.dt.float32

    nc = bass.Bass()
    XT = nc.declare_dram_parameter("xt", [DIM, NC_N], f16, isOutput=False)
    W0 = nc.declare_dram_parameter("w0", [512, 512], f16, isOutput=False)
    W1 = nc.declare_dram_parameter("w1", [256, 256], f16, isOutput=False)
    W2 = nc.declare_dram_parameter("w2", [128, 128], f16, isOutput=False)
    YT = nc.declare_dram_parameter("yt", [DIM, NC_N], f16, isOutput=True)

    TC = _chunked_drain_tile_context(tile, mybir)
    n_nchunks = NC_N // NCHUNK

    with TC(nc) as tc:
        with (
            tc.tile_pool(name="w", bufs=1) as wpool,
            tc.tile_pool(name="x", bufs=1) as xpool,
            tc.tile_pool(name="o", bufs=4) as opool,
            tc.tile_pool(name="ps", bufs=8, space="PSUM") as pspool,
        ):
            # --- weights: resident, one tile per 128-row u-chunk ---
            w0t = []
            for uc in range(4):
                t = wpool.tile([P, 512], f16, tag=f"w0_{uc}")
                nc.sync.dma_start(out=t[:], in_=W0[uc * P : (uc + 1) * P, :])
                w0t.append(t)
            w1t = []
            for uc in range(2):
                t = wpool.tile([P, 256], f16, tag=f"w1_{uc}")
                nc.sync.dma_start(out=t[:], in_=W1[uc * P : (uc + 1) * P, :])
                w1t.append(t)
            w2t = wpool.tile([P, 128], f16, tag="w2")
            nc.sync.dma_start(out=w2t[:], in_=W2[:, :])

            # --- x: one resident [128, 15, 2048] tile, loaded in ~1.3MB DMAs
            # grouped to unblock segment 0 compute earliest ---
            n_blocks = DIM // P
            xall = xpool.tile([P, n_blocks, NC_N], f16, tag="xall")
            xt_blocked = XT.rearrange("(c p) n -> p c n", p=P)
            for b0, b1 in ((0, 2), (2, 4), (4, 7), (7, 10), (10, 13), (13, 15)):
                nc.sync.dma_start(
                    out=xall[:, b0:b1, :], in_=xt_blocked[:, b0:b1, :]
                )
            xtiles = [xall[:, b, :] for b in range(n_blocks)]

            def emit(out_row, w_tiles, w_col0, x_blocks):
                """yt[out_row:out_row+128] = sum_k w_tiles[k][:, wc].T @ x_blocks[k],
                staged in SBUF across all n-chunks, one contiguous 512KB write."""
                stage = opool.tile([P, NC_N], f16, tag="o")
                for j in range(n_nchunks):
                    ps = pspool.tile([P, NCHUNK], f32, tag="ps")
                    nk = len(x_blocks)
                    for k in range(nk):
                        nc.tensor.matmul(
                            ps[:],
                            w_tiles[k][:, w_col0 : w_col0 + P],
                            x_blocks[k][:, j * NCHUNK : (j + 1) * NCHUNK],
                            start=(k == 0),
                            stop=(k == nk - 1),
                        )
                    nc.vector.tensor_copy(
                        out=stage[:, j * NCHUNK : (j + 1) * NCHUNK], in_=ps[:]
                    )
                nc.scalar.dma_start(out=YT[out_row : out_row + P, :], in_=stage[:])

            for _rep in range(reps):
                # segment 0: rows v in [0, 512)
                for vc in range(4):
                    emit(vc * P, w0t, vc * P, xtiles[0:4])
                # segment 1: rows 512 + i*256 + v
                for i in range(3):
                    xb = xtiles[4 + 2 * i : 4 + 2 * i + 2]
                    for vc in range(2):
                        emit(512 + i * 256 + vc * P, w1t, vc * P, xb)
                # segment 2: rows 1280 + i*128 + v
                for i in range(5):
                    emit(1280 + i * P, [w2t], 0, [xtiles[10 + i]])

    _split_multiwait(nc, mybir)
    return nc


class _SpmdRunner:
    def __init__(self, nc, n_cores):
        import jax
        from jax.sharding import Mesh, PartitionSpec
        from jax.experimental.shard_map import shard_map
        from concourse import mybir
        from concourse.bass2jax import (
            _bass_exec_p,
            install_neuronx_cc_hook,
            partition_id_tensor,
        )

        install_neuronx_cc_hook()
        self.jax = jax
        self.n_cores = n_cores
        partition_name = (
            nc.partition_id_tensor.name if nc.partition_id_tensor else None
        )
        in_names, out_names, out_avals = [], [], []
        for alloc in nc.m.functions[0].allocations:
            if not isinstance(alloc, mybir.MemoryLocationSet):
                continue
            name = alloc.memorylocations[0].name
            if alloc.kind == "ExternalInput":
                if name != partition_name:
                    in_names.append(name)
            elif alloc.kind == "ExternalOutput":
                out_names.append(name)
                out_avals.append(
                    jax.core.ShapedArray(
                        tuple(alloc.tensor_shape), mybir.dt.np(alloc.dtype)
                    )
                )
        self.in_names = in_names
        self.out_names = out_names
        self.out_avals = out_avals
        self.n_params = len(in_names)
        all_in_names = in_names + out_names
        if partition_name is not None:
            all_in_names = all_in_names + [partition_name]

        def _body(*args):
            operands = list(args)
            if partition_name is not None:
                operands.append(partition_id_tensor())
            outs = _bass_exec_p.bind(
                *operands,
                out_avals=tuple(out_avals),
                in_names=tuple(all_in_names),
                out_names=tuple(out_names),
                lowering_input_output_aliases=(),
                sim_require_finite=True,
                sim_require_nnan=True,
                nc=nc,
            )
            return tuple(outs)

        devices = jax.devices()[:n_cores]
        self.mesh = Mesh(np.asarray(devices), ("core",))
        n_args = self.n_params + len(out_names)
        self.fn = jax.jit(
            shard_map(
                _body,
                mesh=self.mesh,
                in_specs=(PartitionSpec("core"),) * n_args,
                out_specs=(PartitionSpec("core"),) * len(out_names),
                check_rep=False,
            ),
            keep_unused=True,
        )
        self._dev_args = None

    def set_inputs(self, in_maps):
        import jax
        from jax.sharding import PartitionSpec

        per_core = [[np.asarray(m[name]) for name in self.in_names] for m in in_maps]
        concat_in = [
            np.concatenate([per_core[c][i] for c in range(self.n_cores)], axis=0)
            for i in range(self.n_params)
        ]
        concat_zeros = [
            np.zeros((self.n_cores * a.shape[0], *a.shape[1:]), a.dtype)
            for a in self.out_avals
        ]
        sharding = jax.sharding.NamedSharding(self.mesh, PartitionSpec("core"))
        self._dev_args = [
            jax.device_put(a, sharding) for a in (*concat_in, *concat_zeros)
        ]

    def run_raw(self):
        return self.fn(*self._dev_args)

    def run(self):
        out_arrs = self.jax.block_until_ready(self.run_raw())
        return [
            {
                name: np.asarray(out_arrs[i]).reshape(
                    self.n_cores, *self.out_avals[i].shape
                )[c]
                for i, name in enumerate(self.out_names)
            }
            for c in range(self.n_cores)
        ]


def _get_runner():
    global _runner
    if _runner is None:
        _runner = _SpmdRunner(_build_nc(), N_CORES)
    return _runner


def _pack_x(x):
    """[N, 1920] f32 -> blocked channel-major [1920, N] f16."""
    n = x.shape[0]
    x0 = x[:, :512].T
    x1 = x[:, 512:1280].reshape(n, 256, 3).transpose(2, 1, 0).reshape(768, n)
    x2 = x[:, 1280:1920].reshape(n, 128, 5).transpose(2, 1, 0).reshape(640, n)
    return np.concatenate([x0, x1, x2], axis=0).astype(np.float16)


def _unpack_y(yt):
    """blocked [1920, N] f16 -> [N, 1920] f32."""
    n = yt.shape[1]
    y0 = yt[:512].T
    y1 = yt[512:1280].reshape(3, 256, n).transpose(2, 1, 0).reshape(n, 768)
    y2 = yt[1280:1920].reshape(5, 128, n).transpose(2, 1, 0).reshape(n, 640)
    return np.concatenate([y0, y1, y2], axis=1).astype(np.float32)


def _pack_weights(weight):
    w = np.asarray(weight, dtype=np.float32)
    out = {}
    off = 0
    for idx, (mul, _d) in enumerate(IRREPS):
        blk = w[off : off + mul * mul].reshape(mul, mul) / np.sqrt(np.float32(mul))
        out[f"w{idx}"] = blk.astype(np.float16)
        off += mul * mul
    return out


def kernel(x, weight):
    x = np.asarray(x)
    runner = _get_runner()
    xt = _pack_x(x)
    wmap = _pack_weights(weight)
    in_maps = []
    for c in range(N_CORES):
        m = {"xt": np.ascontiguousarray(xt[:, c * NC_N : (c + 1) * NC_N])}
        m.update(wmap)
        in_maps.append(m)
    runner.set_inputs(in_maps)
    results = runner.run()
    yt = np.concatenate([results[c]["yt"] for c in range(N_CORES)], axis=1)
    return _unpack_y(yt)


# revision 11
# speedup vs baseline: 103.8120x; 101.6366x over previous
"""Segmented (block-diagonal per-irrep) linear layer on 8 TRN2 NeuronCores.

Strategy: data-parallel over rows (N=16384 -> 2048/core). Host pre-transposes
x into a channel-major blocked layout so every device matmul is natural:
  yT[v, n] = sum_u Wseg[u, v] * xT[u, n]   (per irrep component)
with weights stationary [K=u, M=v], x moving [K=u, N=n], fp16 storage/compute
(scale 1/sqrt(mul) pre-folded into weights in fp32), fp32 PSUM accumulation.
"""
import sys

sys.path.insert(0, "/opt/trn_rl_repo")

import numpy as np

IRREPS = [(512, 1), (256, 3), (128, 5)]
N_TOTAL = 16384
N_CORES = 8
NC_N = N_TOTAL // N_CORES          # 2048 rows per core
DIM = 1920
NCHUNK = 512                        # matmul moving free dim
P = 128

_runner = None


def _chunked_drain_tile_context(tile, mybir, max_waits=1):
    """TileContext whose final drain splits sem waits across nops.

    The walrus build in this container rejects >2 sync waits on one
    instruction ("Too many sync wait commands"); stock Tile attaches every
    outstanding sem wait to the single kernel-tail Drain. Equivalent
    semantics: chain of same-queue nops each carrying <=2 waits.
    """
    from concourse.vector_clock import ScopedClock

    class ChunkedDrainTileContext(tile.TileContext):
        def _drain_and_barrier(self, tick_clock, wait_clock):
            probe = self.nc.sync.nop()
            wait_clock.add_sem_waits(
                probe.ins, ScopedClock({None: tick_clock.global_clock})
            )
            waits = list(probe.ins.sync_info.on_wait) if probe.ins.sync_info else []
            probe.ins.sync_info = mybir.SyncInfo(
                on_wait=waits[:max_waits], on_update=[]
            )
            for i in range(max_waits, len(waits), max_waits):
                n = self.nc.sync.nop()
                n.ins.sync_info = mybir.SyncInfo(
                    on_wait=waits[i : i + max_waits], on_update=[]
                )
            self.nc.sync.drain()
            self.nc.all_engine_barrier()
            assert self.sems is not None
            popped = self.nc._tile_sem_poison_stack.pop()
            assert popped is self._sem_poison
            self.nc.clear_and_free_semaphores(list(self.sems.allocated().values()))
            self.nc.all_engine_barrier()

    return ChunkedDrainTileContext


def _split_multiwait(nc, mybir, max_waits=1):
    """Walrus in this container rejects >2 sync waits per instruction.

    Move excess waits onto freshly inserted NoOps just before the
    instruction on the same engine queue — identical sync semantics.
    """
    seq = 0
    for f in nc.m.functions:
        for blk in f.blocks:
            changed = False
            new = []
            for inst in blk.instructions:
                si = inst.sync_info
                waits = list(si.on_wait) if si else []
                if len(waits) > max_waits:
                    changed = True
                    updates = list(si.on_update)
                    extra = waits[:-max_waits]
                    for i in range(0, len(extra), max_waits):
                        nop = mybir.InstNoOp(
                            name=f"I-waitsplit-{seq}", ins=[], outs=[]
                        )
                        seq += 1
                        nop.engine = inst.engine
                        nop.sync_info = mybir.SyncInfo(
                            on_wait=extra[i : i + max_waits], on_update=[]
                        )
                        new.append(nop)
                    inst.sync_info = mybir.SyncInfo(
                        on_wait=waits[-max_waits:], on_update=updates
                    )
                new.append(inst)
            if changed:
                blk.instructions = new


def _build_nc(reps=1):
    import concourse.bass as bass
    import concourse.tile as tile
    from concourse import mybir

    f16 = mybir.dt.float16
    f32 = mybir.dt.float32

    nc = bass.Bass()
    XT = nc.declare_dram_parameter("xt", [DIM, NC_N], f16, isOutput=False)
    W0 = nc.declare_dram_parameter("w0", [512, 512], f16, isOutput=False)
    W1 = nc.declare_dram_parameter("w1", [256, 256], f16, isOutput=False)
    W2 = nc.declare_dram_parameter("w2", [128, 128], f16, isOutput=False)
    YT = nc.declare_dram_parameter("yt", [DIM, NC_N], f16, isOutput=True)

    TC = _chunked_drain_tile_context(tile, mybir)
    n_nchunks = NC_N // NCHUNK

    with TC(nc) as tc:
        with (
            tc.tile_pool(name="w", bufs=1) as wpool,
            tc.tile_pool(name="x", bufs=1) as xpool,
            tc.tile_pool(name="o", bufs=4) as opool,
            tc.tile_pool(name="ps", bufs=8, space="PSUM") as pspool,
        ):
            # --- weights: resident, one tile per 128-row u-chunk ---
            w0t = []
            for uc in range(4):
                t = wpool.tile([P, 512], f16, tag=f"w0_{uc}")
                nc.sync.dma_start(out=t[:], in_=W0[uc * P : (uc + 1) * P, :])
                w0t.append(t)
            w1t = []
            for uc in range(2):
                t = wpool.tile([P, 256], f16, tag=f"w1_{uc}")
                nc.sync.dma_start(out=t[:], in_=W1[uc * P : (uc + 1) * P, :])
                w1t.append(t)
            w2t = wpool.tile([P, 128], f16, tag="w2")
            nc.sync.dma_start(out=w2t[:], in_=W2[:, :])

            # --- x: one resident [128, 15, 2048] tile, loaded in ~1.3MB DMAs
            # grouped to unblock segment 0 compute earliest ---
            n_blocks = DIM // P
            xall = xpool.tile([P, n_blocks, NC_N], f16, tag="xall")
            xt_blocked = XT.rearrange("(c p) n -> p c n", p=P)
            for b0, b1 in ((0, 2), (2, 4), (4, 7), (7, 10), (10, 13), (13, 15)):
                nc.sync.dma_start(
                    out=xall[:, b0:b1, :], in_=xt_blocked[:, b0:b1, :]
                )
            xtiles = [xall[:, b, :] for b in range(n_blocks)]

            def emit(out_row, w_tiles, w_col0, x_blocks):
                """yt[out_row:out_row+128] = sum_k w_tiles[k][:, wc].T @ x_blocks[k],
                staged in SBUF across all n-chunks, one contiguous 512KB write."""
                stage = opool.tile([P, NC_N], f16, tag="o")
                for j in range(n_nchunks):
                    ps = pspool.tile([P, NCHUNK], f32, tag="ps")
                    nk = len(x_blocks)
                    for k in range(nk):
                        nc.tensor.matmul(
                            ps[:],
                            w_tiles[k][:, w_col0 : w_col0 + P],
                            x_blocks[k][:, j * NCHUNK : (j + 1) * NCHUNK],
                            start=(k == 0),
                            stop=(k == nk - 1),
                        )
                    nc.vector.tensor_copy(
                        out=stage[:, j * NCHUNK : (j + 1) * NCHUNK], in_=ps[:]
                    )
                nc.scalar.dma_start(out=YT[out_row : out_row + P, :], in_=stage[:])

            for _rep in range(reps):
                # segment 0: rows v in [0, 512)
                for vc in range(4):
                    emit(vc * P, w0t, vc * P, xtiles[0:4])
                # segment 1: rows 512 + i*256 + v
                for i in range(3):
                    xb = xtiles[4 + 2 * i : 4 + 2 * i + 2]
                    for vc in range(2):
                        emit(512 + i * 256 + vc * P, w1t, vc * P, xb)
                # segment 2: rows 1280 + i*128 + v
                for i in range(5):
                    emit(1280 + i * P, [w2t], 0, [xtiles[10 + i]])

    _split_multiwait(nc, mybir)
    return nc


class _SpmdRunner:
    def __init__(self, nc, n_cores):
        import jax
        from jax.sharding import Mesh, PartitionSpec
        from jax.experimental.shard_map import shard_map
        from concourse import mybir
        from concourse.bass2jax import (
            _bass_exec_p,
            install_neuronx_cc_hook,
            partition_id_tensor,
        )

        install_neuronx_cc_hook()
        self.jax = jax
        self.n_cores = n_cores
        partition_name = (
            nc.partition_id_tensor.name if nc.partition_id_tensor else None
        )
        in_names, out_names, out_avals = [], [], []
        for alloc in nc.m.functions[0].allocations:
            if not isinstance(alloc, mybir.MemoryLocationSet):
                continue
            name = alloc.memorylocations[0].name
            if alloc.kind == "ExternalInput":
                if name != partition_name:
                    in_names.append(name)
            elif alloc.kind == "ExternalOutput":
                out_names.append(name)
                out_avals.append(
                    jax.core.ShapedArray(
                        tuple(alloc.tensor_shape), mybir.dt.np(alloc.dtype)
                    )
                )
        self.in_names = in_names
        self.out_names = out_names
        self.out_avals = out_avals
        self.n_params = len(in_names)
        all_in_names = in_names + out_names
        if partition_name is not None:
            all_in_names = all_in_names + [partition_name]

        def _body(*args):
            operands = list(args)
            if partition_name is not None:
                operands.append(partition_id_tensor())
            outs = _bass_exec_p.bind(
                *operands,
                out_avals=tuple(out_avals),
                in_names=tuple(all_in_names),
                out_names=tuple(out_names),
                lowering_input_output_aliases=(),
                sim_require_finite=True,
                sim_require_nnan=True,
                nc=nc,
            )
            return tuple(outs)

        devices = jax.devices()[:n_cores]
        self.mesh = Mesh(np.asarray(devices), ("core",))
        n_args = self.n_params + len(out_names)
        self.fn = jax.jit(
            shard_map(
                _body,
                mesh=self.mesh,
                in_specs=(PartitionSpec("core"),) * n_args,
                out_specs=(PartitionSpec("core"),) * len(out_names),
                check_rep=False,
            ),
            keep_unused=True,
        )
        self._dev_args = None

    def set_inputs(self, in_maps):
        import jax
        from jax.sharding import PartitionSpec

        per_core = [[np.asarray(m[name]) for name in self.in_names] for m in in_maps]
        concat_in = [
            np.concatenate([per_core[c][i] for c in range(self.n_cores)], axis=0)
            for i in range(self.n_params)
        ]
        concat_zeros = [
            np.zeros((self.n_cores * a.shape[0], *a.shape[1:]), a.dtype)
            for a in self.out_avals
        ]
        sharding = jax.sharding.NamedSharding(self.mesh, PartitionSpec("core"))
        self._dev_args = [
            jax.device_put(a, sharding) for a in (*concat_in, *concat_zeros)
        ]

    def run_raw(self):
        return self.fn(*self._dev_args)

    def run(self):
        out_arrs = self.jax.block_until_ready(self.run_raw())
        return [
            {
                name: np.asarray(out_arrs[i]).reshape(
                    self.n_cores, *self.out_avals[i].shape
                )[c]
                for i, name in enumerate(self.out_names)
            }
            for c in range(self.n_cores)
        ]


def _get_runner():
    global _runner
    if _runner is None:
        _runner = _SpmdRunner(_build_nc(), N_CORES)
    return _runner


def _pack_x(x):
    """[N, 1920] f32 -> blocked channel-major [1920, N] f16."""
    n = x.shape[0]
    x0 = x[:, :512].T
    x1 = x[:, 512:1280].reshape(n, 256, 3).transpose(2, 1, 0).reshape(768, n)
    x2 = x[:, 1280:1920].reshape(n, 128, 5).transpose(2, 1, 0).reshape(640, n)
    return np.concatenate([x0, x1, x2], axis=0).astype(np.float16)


def _unpack_y(yt):
    """blocked [1920, N] f16 -> [N, 1920] f32."""
    n = yt.shape[1]
    y0 = yt[:512].T
    y1 = yt[512:1280].reshape(3, 256, n).transpose(2, 1, 0).reshape(n, 768)
    y2 = yt[1280:1920].reshape(5, 128, n).transpose(2, 1, 0).reshape(n, 640)
    return np.concatenate([y0, y1, y2], axis=1).astype(np.float32)


def _pack_weights(weight):
    w = np.asarray(weight, dtype=np.float32)
    out = {}
    off = 0
    for idx, (mul, _d) in enumerate(IRREPS):
        blk = w[off : off + mul * mul].reshape(mul, mul) / np.sqrt(np.float32(mul))
        out[f"w{idx}"] = blk.astype(np.float16)
        off += mul * mul
    return out


def kernel(x, weight):
    x = np.asarray(x)
    runner = _get_runner()
    xt = _pack_x(x)
    wmap = _pack_weights(weight)
    in_maps = []
    for c in range(N_CORES):
        m = {"xt": np.ascontiguousarray(xt[:, c * NC_N : (c + 1) * NC_N])}
        m.update(wmap)
        in_maps.append(m)
    runner.set_inputs(in_maps)
    results = runner.run()
    yt = np.concatenate([results[c]["yt"] for c in range(N_CORES)], axis=1)
    return _unpack_y(yt)
